# revision 21
# baseline (speedup 1.0000x reference)
"""Trainium2 Bass kernel: full encoder-decoder transformer decoder layer.

Contract: kernel(**inputs) takes FULL unsharded inputs and returns the
FULL [B, T, D] float32 output.

Sharding: pure data-parallel over (batch, T-half) -> 8 cores, zero
collectives.  Each core computes TL=1024 decoder rows end-to-end; the
full-T K/V projections are computed redundantly by the 2 cores sharing
a batch element.

v2 design (vs. the bf16 baseline):
 - fp8(e4m3) + DoubleRow matmuls for every attention-side projection
   (QKV, cross KV/Q, both out-projs).  Weights are host-scaled by 64
   into the fp8 dynamic range; evictions descale by 1/4096.  The FFN
   stays bf16 (its output is ~half of the final LN input; fp8's ~5%
   dot-product noise would break the tolerance there).
 - Software-pipelined schedule: the two attention phases are
   exp(ScalarE)-bound, so independent projection matmuls (cross-KV
   during self-attn, FFN during cross-attn) are interleaved into their
   PE idle slots via a filler queue.  Tile's dependency tracking keeps
   any interleaving correct.
 - Softmax normalization batched per q-chunk: denominators (from the
   appended ones column of V) are gathered into one [H, QC] tile, one
   fast-approx reciprocal instead of 64 slow ones, and a batched
   DRAM-bounce partition-broadcast.
 - res1/res2 live in SBUF as bf16 (no DRAM spills).
"""

from collections import deque
from contextlib import ExitStack

import ml_dtypes
import numpy as np

import concourse.bass as bass
import concourse.mybir as mybir
import concourse.tile as tile
from concourse import bacc
from concourse.bass_utils import run_bass_kernel_spmd

P = 128
HD = 64  # head dim (fixed)
BF = mybir.dt.bfloat16
F8 = mybir.dt.float8e4
F32 = mybir.dt.float32
AF = mybir.ActivationFunctionType
ALU = mybir.AluOpType
DR = mybir.MatmulPerfMode.DoubleRow
EPS = 1e-5
VP = 80            # padded per-head V stride (fp8 alignment: 80 = 16-aligned)
WS = 64.0          # fp8 weight scale
INV2 = 1.0 / (WS * WS)
DEBUG = {"stage": None}   # None | "A" | "attn" | "ln1"


def build_program(D=1024, H=16, T=2048, TL=1024, S=2048, DFF=4096, loop_n=1):
    assert D == H * HD
    KT = D // P
    KP = KT // 2
    NKT = T // P
    NSK = S // P
    FT = DFF // P
    HP = H // 2
    QC = min(512, TL)
    NQ = TL // QC

    nc = bacc.Bacc()
    tens = {}

    def din(name, shape, dtype=F8):
        tens[name] = nc.declare_dram_parameter(name, list(shape), dtype,
                                               isOutput=False)
        return tens[name]

    din("xdT", (KP, P, 2, T))
    din("xqT", (KP, P, 2, TL))
    din("xeT", (KP, P, 2, S))
    din("xres", (KT, P, TL), F32)
    for nm in ("wq", "wk", "wv", "wo1", "wqc", "wkc", "wvc", "wo2"):
        din(nm, (KP, P, 2, D))
    din("w1", (KT, P, DFF), BF)
    din("w2", (FT, P, D), BF)
    for nm in ("bq", "bk", "bqc", "bkc", "bo2", "b2f",
               "g1", "be1", "g2", "be2", "g3", "be3"):
        din(nm, (KT, P, 1), F32)
    din("b1f", (FT, P, 1), F32)
    din("bv_row", (1, D), F32)
    din("bvc_row", (1, D), F32)

    tens["outT"] = nc.declare_dram_parameter("outT", [KT, P, TL], F32,
                                             isOutput=True)
    tens["r_bounce"] = nc.dram_tensor("r_bounce", [2, NQ, H, QC], F32)
    if DEBUG["stage"] is not None:
        tens["dbg_k"] = nc.declare_dram_parameter(
            "dbg_k", [HP, P, T], F8, isOutput=True)
        tens["dbg_q"] = nc.declare_dram_parameter(
            "dbg_q", [HP, P, TL], F8, isOutput=True)
        tens["dbg_v"] = nc.declare_dram_parameter(
            "dbg_v", [NKT, P, H * VP], F8, isOutput=True)
        tens["dbg_mg"] = nc.declare_dram_parameter(
            "dbg_mg", [KP, P, 2 * TL], F8, isOutput=True)
        tens["dbg_z1"] = nc.declare_dram_parameter(
            "dbg_z1", [KT, P, QC], F32, isOutput=True)
        tens["dbg_z2"] = nc.declare_dram_parameter(
            "dbg_z2", [KT, P, QC], F32, isOutput=True)
        tens["dbg_mgc"] = nc.declare_dram_parameter(
            "dbg_mgc", [KP, P, 2 * TL], F8, isOutput=True)
        tens["dbg_r1"] = nc.declare_dram_parameter(
            "dbg_r1", [KT, P, TL], BF, isOutput=True)
        tens["dbg_sc"] = nc.declare_dram_parameter(
            "dbg_sc", [P, 2 * QC], F32, isOutput=True)
        tens["dbg_ex"] = nc.declare_dram_parameter(
            "dbg_ex", [P, 2 * QC], F8, isOutput=True)
        tens["dbg_av"] = nc.declare_dram_parameter(
            "dbg_av", [P, QC], F32, isOutput=True)

    cfg = dict(D=D, H=H, T=T, TL=TL, S=S, DFF=DFF, KT=KT, KP=KP, NKT=NKT,
               NSK=NSK, FT=FT, HP=HP, QC=QC, NQ=NQ, tens=tens)

    with tile.TileContext(nc) as tc:
        if loop_n > 1:
            with tc.For_i(0, loop_n, 1) as _i:
                _build(tc, cfg)
        else:
            _build(tc, cfg)

    nc.finalize()
    return nc


class Filler:
    """Queue of generators; each step() emits one PE-quantum of work."""

    def __init__(self):
        self.q = deque()

    def push(self, gen):
        self.q.append(gen)

    def step(self, n=1):
        for _ in range(n):
            while self.q:
                try:
                    next(self.q[0])
                    break
                except StopIteration:
                    self.q.popleft()
            else:
                return

    def drain(self):
        while self.q:
            self.step()


def _build(tc, cfg):
    nc = tc.nc
    D, H, T, TL, S, DFF = (cfg["D"], cfg["H"], cfg["T"], cfg["TL"], cfg["S"],
                           cfg["DFF"])
    KT, KP, NKT, NSK, FT, HP, QC, NQ = (
        cfg["KT"], cfg["KP"], cfg["NKT"], cfg["NSK"], cfg["FT"], cfg["HP"],
        cfg["QC"], cfg["NQ"])
    tens = cfg["tens"]

    def dram(name):
        return tens[name][:]

    ctx = ExitStack()
    # ---------------- global pools ----------------
    const = ctx.enter_context(tc.tile_pool(name="const", bufs=1))
    # PSUM: scores 2x[128,2QC] (4 banks) + AV 2x[128,QC] (2) + proj 2x (2)
    scp = ctx.enter_context(tc.tile_pool(name="scp", bufs=2, space="PSUM"))
    avp = ctx.enter_context(tc.tile_pool(name="avp", bufs=1, space="PSUM"))
    acc = ctx.enter_context(tc.tile_pool(name="acc", bufs=2, space="PSUM"))
    smp = ctx.enter_context(tc.tile_pool(name="smp", bufs=2))
    expp = ctx.enter_context(tc.tile_pool(name="expp", bufs=3))
    wstr = ctx.enter_context(tc.tile_pool(name="wstr", bufs=2))
    znp = ctx.enter_context(tc.tile_pool(name="znp", bufs=1))
    xrp = ctx.enter_context(tc.tile_pool(name="xrp", bufs=1))

    # ---------------- constants ----------------
    ones_bf = const.tile([P, P], BF, tag="ones_bf", name="ones_bf")
    nc.vector.memset(ones_bf[:], 1.0)
    eps_t = const.tile([P, 1], F32, tag="eps_t", name="eps_t")
    nc.vector.memset(eps_t[:], EPS)

    def load_pp(name, n):
        out = []
        src = dram(name)
        for j in range(n):
            tl_ = const.tile([P, 1], F32, tag=f"{name}{j}", name=f"{name}{j}")
            nc.sync.dma_start(out=tl_[:], in_=src[j])
            out.append(tl_)
        return out

    bq = load_pp("bq", KT); bk = load_pp("bk", KT)
    bqc = load_pp("bqc", KT); bkc = load_pp("bkc", KT)
    bo2 = load_pp("bo2", KT); b2f = load_pp("b2f", KT)
    b1f = load_pp("b1f", FT)
    g1 = load_pp("g1", KT); be1 = load_pp("be1", KT)
    g2 = load_pp("g2", KT); be2 = load_pp("be2", KT)
    g3 = load_pp("g3", KT); be3 = load_pp("be3", KT)

    def bias_bcast(name):
        tl_ = const.tile([P, D], BF, tag=f"{name}_bc", name=f"{name}_bc")
        src = dram(name)
        bcast_ap = bass.AP(tensor=src.tensor, offset=0, ap=[[0, P], [1, D]])
        nc.gpsimd.dma_start(out=tl_[:], in_=bcast_ap)
        return tl_

    vb_bc = bias_bcast("bv_row")
    vcb_bc = bias_bcast("bvc_row")

    # ---------------- persistent activations (allocated upfront) --------
    big = ctx.enter_context(tc.tile_pool(name="big", bufs=1))

    def mk(pool, prefix, n, shape, dtype):
        return [pool.tile(list(shape), dtype, tag=f"{prefix}{i}",
                          name=f"{prefix}{i}") for i in range(n)]

    KTc = mk(big, "KTc", HP, (P, S), F8)
    QTc = mk(big, "QTc", HP, (P, TL), F8)
    Vsc = mk(big, "Vsc", NSK, (P, H * VP), F8)
    mg = mk(big, "mg", KP, (P, 2 * TL), F8)     # merged^T x64, both attns
    res1b = mk(big, "res1b", KT, (P, TL), BF)
    res1f8 = mk(big, "res1f8", KP, (P, 2 * TL), F8)

    def pair_view(t, span):
        return t[:].rearrange("p (two t) -> p two t", two=2)

    # =====================================================================
    # fp8 DoubleRow projection (generator)
    # =====================================================================
    ACCG = 2

    INV1 = 1.0 / WS

    def projT_dr(wname, x_views, nF, evict, tagset, csl=None, Tlen=None):
        C = 512
        if csl is None:
            tcis = [(i, slice(i * C, (i + 1) * C)) for i in range(Tlen // C)]
        else:
            tcis = [csl]
        nfj = nF // P
        w = dram(wname)
        for fg in range((nfj + ACCG - 1) // ACCG):
            js = list(range(fg * ACCG, min((fg + 1) * ACCG, nfj)))
            wts = []
            for j in range(KP):
                wt = wstr.tile([P, 2 * len(js) * P], F8, tag=f"w{tagset}{j}",
                               name=f"w_{wname}_{j}",
                               bufs=1 if tagset == "F" else 2)
                nc.sync.dma_start(
                    out=wt[:],
                    in_=w[j][:, :, js[0] * P:(js[-1] + 1) * P])
                wts.append(wt)
            yield
            for tci, cs in tcis:
                for jj in range(len(js)):
                    ps = acc.tile([P, C], F32, tag="acc", name="acc_ps")
                    for j in range(KP):
                        wv_ = wts[j][:].rearrange("p (two f) -> p two f",
                                                  two=2)
                        nc.tensor.matmul(
                            ps[:],
                            lhsT=wv_[:, :, jj * P:(jj + 1) * P],
                            rhs=x_views[j][:, :, cs],
                            start=(j == 0), stop=(j == KP - 1),
                            perf_mode=DR)
                        yield
                    evict(js[jj], tci, ps)

    def v_proj_dr(x_pairs, w_tiles, Vdst, nkt, vbias_bc):
        NVJ = D // 512
        VC = 512
        for kt in range(nkt):
            vsl = Vdst[kt][:].rearrange("p (h c) -> p h c", c=VP)
            nc.vector.memset(vsl[:, :, HD:VP], 1.0)
            for vj in range(NVJ):
                ps = acc.tile([P, VC], F32, tag="acc", name="v_ps")
                for j in range(KP):
                    wv_ = w_tiles[j][:].rearrange("p (two f) -> p two f",
                                                  two=2)
                    nc.tensor.matmul(
                        ps[:], lhsT=x_pairs[j][:, :, kt * P:(kt + 1) * P],
                        rhs=wv_[:, :, vj * VC:(vj + 1) * VC],
                        start=(j == 0), stop=(j == KP - 1), perf_mode=DR)
                    yield
                nc.vector.scalar_tensor_tensor(
                    out=vsl[:, (VC // HD) * vj:(VC // HD) * (vj + 1), 0:HD],
                    in0=ps[:], scalar=1.0 / WS,
                    in1=vbias_bc[:, vj * VC:(vj + 1) * VC],
                    op0=ALU.mult, op1=ALU.add)

    # =====================================================================
    # attention chunk (fp8 operands); fills PE slack from `filler`
    # =====================================================================
    def attn_chunk(KTt_, QTt_, Vst_, nkt, aidx, qi, filler, fill_n):
        qsl = slice(qi * QC, (qi + 1) * QC)
        rb = tens["r_bounce"][aidx, qi]
        for jt in range(HP):
            avE = avp.tile([P, QC], F32, tag="avE", name="avE")
            avO = avp.tile([P, QC], F32, tag="avO", name="avO")
            for kt in range(nkt):
                sc = scp.tile([P, 2 * QC], F32, tag="sc", name="sc_ps")
                ksl = slice(kt * P, (kt + 1) * P)
                nc.tensor.matmul(
                    sc[:, 0:QC],
                    lhsT=KTt_[jt][0:HD, ksl], rhs=QTt_[jt][0:HD, qsl],
                    start=True, stop=True, tile_position=(0, 0))
                nc.tensor.matmul(
                    sc[:, QC:2 * QC],
                    lhsT=KTt_[jt][HD:P, ksl], rhs=QTt_[jt][HD:P, qsl],
                    start=True, stop=True, tile_position=(HD, 0))
                ex = expp.tile([P, 2 * QC], F8, tag="ex", name="ex")
                nc.scalar.activation(ex[:], sc[:], AF.Exp, scale=0.125)
                if (DEBUG["stage"] == "attnraw" and aidx == 0 and jt == 0
                        and kt == 0 and qi == NQ - 1):
                    for hh in range(2):
                        sccp = znp.tile([P, QC], F32, tag="s0", name="sccp")
                        nc.vector.tensor_copy(
                            out=sccp[:], in_=sc[:, hh * QC:(hh + 1) * QC])
                        nc.sync.dma_start(
                            out=dram("dbg_sc")[:, hh * QC:(hh + 1) * QC],
                            in_=sccp[:])
                    nc.sync.dma_start(out=dram("dbg_ex"), in_=ex[:])
                vsl = Vst_[kt][:].rearrange("p (h c) -> p h c", c=VP)
                nc.tensor.matmul(
                    avE[0:VP, :], lhsT=vsl[:, 2 * jt, :],
                    rhs=ex[:, 0:QC],
                    start=(kt == 0), stop=(kt == nkt - 1))
                nc.tensor.matmul(
                    avO[0:VP, :], lhsT=vsl[:, 2 * jt + 1, :],
                    rhs=ex[:, QC:2 * QC],
                    start=(kt == 0), stop=(kt == nkt - 1))
                filler.step(fill_n)
            j, i = jt // 2, jt % 2
            if (DEBUG["stage"] == "attnraw" and aidx == 0 and jt == 0
                    and qi == NQ - 1):
                avcp = znp.tile([P, QC], F32, tag="s0", name="avcp")
                nc.vector.tensor_copy(out=avcp[:], in_=avE[:, :])
                nc.sync.dma_start(out=dram("dbg_av"), in_=avcp[:])
            mgv = pair_view(mg[j], TL)
            for av, e in ((avE, 0), (avO, 1)):
                h = 2 * jt + e
                nc.vector.tensor_scalar_mul(
                    mgv[e * HD:(e + 1) * HD, i, qsl], av[0:HD, :], 0.125)
                rh = smp.tile([1, QC], F32, tag="rh", name="rh", bufs=2)
                nc.vector.reciprocal(rh[:], av[HD:HD + 1, :])
                nc.sync.dma_start(out=rb[h], in_=rh[:])

    def norm_chunk(aidx, qi):
        qsl = slice(qi * QC, (qi + 1) * QC)
        rb = tens["r_bounce"][aidx, qi]
        for jt in range(HP):
            j, i = jt // 2, jt % 2
            mgv = pair_view(mg[j], TL)
            bc = smp.tile([P, QC], F32, tag="bcast", name="bcast")
            for e in range(2):
                rslot = rb[2 * jt + e]
                r_bcast = bass.AP(tensor=rslot.tensor, offset=rslot.offset,
                                  ap=[[0, HD]] + list(rslot.ap[-1:]))
                nc.gpsimd.dma_start(out=bc[e * HD:(e + 1) * HD, :],
                                    in_=r_bcast)
            for e in range(2):
                sl_ = mgv[e * HD:(e + 1) * HD, i, qsl]
                nc.vector.scalar_tensor_tensor(
                    out=sl_, in0=sl_, scalar=8.0 * WS,
                    in1=bc[e * HD:(e + 1) * HD, :],
                    op0=ALU.mult, op1=ALU.mult)

    # =====================================================================
    # layernorm chunk (reduce over D = partition dim, transposed layout)
    # =====================================================================
    def layer_norm_chunk(z, zb, g, be, out_sl, f8_sl=None, post_add=None,
                         out_dram=None):
        inv_d = 1.0 / D
        psA = acc.tile([P, QC], F32, tag="acc", name="psA")
        for ki in range(KT):
            nc.tensor.matmul(psA[:], lhsT=ones_bf[:], rhs=zb[ki],
                             start=(ki == 0), stop=(ki == KT - 1))
        psB = acc.tile([P, QC], F32, tag="acc", name="psB")
        for ki in range(KT):
            zq = znp.tile([P, QC], BF, tag=f"zsq{ki % 2}", name="zsq")
            nc.vector.tensor_mul(zq[:], zb[ki], zb[ki])
            nc.tensor.matmul(psB[:], lhsT=ones_bf[:], rhs=zq[:],
                             start=(ki == 0), stop=(ki == KT - 1))
        mean = znp.tile([P, QC], F32, tag="s0", name="mean")
        msq = znp.tile([P, QC], F32, tag="s1", name="msq")
        nc.vector.tensor_scalar_mul(mean[:], psA[:], inv_d)
        nc.vector.tensor_scalar_mul(msq[:], psB[:], inv_d)
        sa = znp.tile([P, QC], F32, tag="s2", name="sa_t")
        nc.vector.tensor_mul(sa[:], mean[:], mean[:])
        nc.vector.tensor_sub(sa[:], msq[:], sa[:])          # var
        sb = znp.tile([P, QC], F32, tag="s3", name="sb_t")
        nc.scalar.activation(sb[:], sa[:], AF.Sqrt, bias=eps_t[:])
        nc.vector.reciprocal(sa[:], sb[:])  # rstd
        nc.vector.tensor_mul(sb[:], mean[:], sa[:])         # mean*rstd
        for ki in range(KT):
            tmp = znp.tile([P, QC], F32, tag="s0", name="lntmp")
            nc.vector.tensor_mul(tmp[:], z[ki], sa[:])
            nc.vector.tensor_sub(tmp[:], tmp[:], sb[:])
            osl = out_sl(ki)
            nc.vector.tensor_scalar(
                out=osl, in0=tmp[:], scalar1=g[ki][:], scalar2=be[ki][:],
                op0=ALU.mult, op1=ALU.add)
            if f8_sl is not None:
                nc.vector.tensor_copy(out=f8_sl(ki), in_=osl)
            if post_add is not None:
                nc.vector.tensor_scalar_add(out=osl, in0=osl,
                                            scalar1=post_add[ki][:])
            if out_dram is not None:
                nc.sync.dma_start(out=out_dram(ki), in_=osl)

    # =====================================================================
    # Phase A: load + self QKV (fp8 DR)
    # =====================================================================
    es_self = ExitStack()
    pself = es_self.enter_context(tc.tile_pool(name="pself", bufs=1))
    KTt = mk(pself, "KTt", HP, (P, T), F8)
    QTt = mk(pself, "QTt", HP, (P, TL), F8)
    Vst = mk(pself, "Vst", NKT, (P, H * VP), F8)

    es_A = ExitStack()
    pA = es_A.enter_context(tc.tile_pool(name="pA", bufs=1))
    xdT = mk(pA, "xdT", KP, (P, 2 * T), F8)
    wv_sb = mk(pA, "wv", KP, (P, 2 * D), F8)
    for j in range(KP):
        nc.sync.dma_start(out=xdT[j][:], in_=dram("xdT")[j])
    for j in range(KP):
        nc.sync.dma_start(out=wv_sb[j][:], in_=dram("wv")[j])

    xdv = [pair_view(t, T) for t in xdT]

    def ev_k(fj, tci, ps):
        nc.vector.tensor_scalar(
            out=KTt[fj][:, tci * 512:(tci + 1) * 512], in0=ps[:],
            scalar1=1.0 / WS, scalar2=bk[fj][:], op0=ALU.mult, op1=ALU.add)

    def ev_q(fj, tci, ps):
        nc.vector.tensor_scalar(
            out=QTt[fj][:, tci * 512:(tci + 1) * 512], in0=ps[:],
            scalar1=1.0 / WS, scalar2=bq[fj][:], op0=ALU.mult, op1=ALU.add)

    fA = Filler()
    fA.push(projT_dr("wk", xdv, D, ev_k, "D", Tlen=T))
    fA.push(v_proj_dr(xdv, wv_sb, Vst, NKT, vb_bc))
    fA.drain()
    es_A.close()

    def dump(name, tiles):
        dst = dram(name)
        for i, t in enumerate(tiles):
            nc.sync.dma_start(out=dst[i], in_=t[:])

    es_A2 = ExitStack()
    pA2 = es_A2.enter_context(tc.tile_pool(name="pA2", bufs=1))
    xqT = mk(pA2, "xqT", KP, (P, 2 * TL), F8)
    for j in range(KP):
        nc.sync.dma_start(out=xqT[j][:], in_=dram("xqT")[j])
    xqv = [pair_view(t, TL) for t in xqT]
    fA2 = Filler()
    fA2.push(projT_dr("wq", xqv, D, ev_q, "D", Tlen=TL))
    fA2.drain()
    es_A2.close()
    if DEBUG["stage"] == "A":
        dump("dbg_k", KTt)
        dump("dbg_q", QTt)
        dump("dbg_v", Vst)
        es_self.close()
        ctx.close()
        return

    # =====================================================================
    # Window 1: self-attn + cross-KV proj (filler) + out1/LN1/crossQ
    # =====================================================================
    es_W1 = ExitStack()
    pW1 = es_W1.enter_context(tc.tile_pool(name="pW1", bufs=1))
    xeT = mk(pW1, "xeT", KP, (P, 2 * S), F8)
    wvc_sb = mk(pW1, "wvc", KP, (P, 2 * D), F8)
    for j in range(KP):
        nc.sync.dma_start(out=xeT[j][:], in_=dram("xeT")[j])
    for j in range(KP):
        nc.sync.dma_start(out=wvc_sb[j][:], in_=dram("wvc")[j])
    xev = [pair_view(t, S) for t in xeT]

    def ev_kc(fj, tci, ps):
        nc.vector.tensor_scalar(
            out=KTc[fj][:, tci * 512:(tci + 1) * 512], in0=ps[:],
            scalar1=1.0 / WS, scalar2=bkc[fj][:], op0=ALU.mult, op1=ALU.add)

    f1 = Filler()
    f1.push(projT_dr("wkc", xev, D, ev_kc, "F", Tlen=S))
    f1.push(v_proj_dr(xev, wvc_sb, Vsc, NSK, vcb_bc))

    res1v = [pair_view(t, TL) for t in res1f8]
    mgv_all = [pair_view(t, TL) for t in mg]

    for qi in range(NQ):
        qsl = slice(qi * QC, (qi + 1) * QC)
        attn_chunk(KTt, QTt, Vst, NKT, 0, qi, f1, 2)
        if DEBUG["stage"] == "attnraw" and qi == NQ - 1:
            f1.drain()
            dump("dbg_mg", mg)
            es_W1.close()
            es_self.close()
            ctx.close()
            return
        norm_chunk(0, qi)
        # out1 chunk -> z1 (f32) + bf16 stats copy
        z1 = [znp.tile([P, QC], F32, tag=f"z_{k}", name="z1")
              for k in range(KT)]
        z1b = [znp.tile([P, QC], BF, tag=f"zb{k}", name="z1b")
              for k in range(KT)]

        def ev_o1(fj, tci, ps, qsl=qsl, z1=z1, z1b=z1b):
            xr = xrp.tile([P, QC], F32, tag="xr", name="xr")
            nc.sync.dma_start(out=xr[:], in_=dram("xres")[fj][:, qsl])
            nc.vector.scalar_tensor_tensor(
                out=z1[fj][:], in0=ps[:], scalar=INV2,
                in1=xr[:], op0=ALU.mult, op1=ALU.add)
            nc.vector.tensor_copy(out=z1b[fj][:], in_=z1[fj][:])

        if DEBUG["stage"] == "attn" and qi == NQ - 1:
            f1.drain()
            dump("dbg_mg", mg)
            es_W1.close()
            es_self.close()
            ctx.close()
            return
        for _ in projT_dr("wo1", mgv_all, D, ev_o1, "D", csl=(0, qsl)):
            pass
        if DEBUG["stage"] == "ln1" and qi == NQ - 1:
            f1.drain()
            dump("dbg_z1", [t[:] for t in z1])
            es_W1.close()
            es_self.close()
            ctx.close()
            return
        layer_norm_chunk(
            [t[:] for t in z1], [t[:] for t in z1b], g1, be1,
            out_sl=lambda ki: res1b[ki][:, qsl],
            f8_sl=lambda ki: res1v[ki // 2][:, ki % 2, qsl],
            post_add=bo2)

        def ev_qc(fj, tci, ps, qsl=qsl):
            nc.vector.tensor_scalar(
                out=QTc[fj][:, qsl], in0=ps[:],
                scalar1=1.0 / WS, scalar2=bqc[fj][:], op0=ALU.mult, op1=ALU.add)

        for _ in projT_dr("wqc", res1v, D, ev_qc, "D", csl=(0, qsl)):
            pass
    f1.drain()
    es_W1.close()
    es_self.close()

    # =====================================================================
    # Window 2: cross-attn + out2/LN2 + FFN (filler) + LN3 -> out
    # =====================================================================
    es_F = ExitStack()
    pF = es_F.enter_context(tc.tile_pool(name="pF", bufs=1))
    hT = mk(pF, "hT", FT, (P, QC), BF)         # chunk-local
    res2b = mk(pF, "res2b", KT, (P, TL), BF)
    w1p = es_F.enter_context(tc.tile_pool(name="w1p", bufs=1))
    w2p = es_F.enter_context(tc.tile_pool(name="w2p", bufs=1))

    f2 = Filler()

    def ffn_chunk(qi):
        qsl = slice(qi * QC, (qi + 1) * QC)
        w1d = dram("w1")
        for fg in range(FT // ACCG):
            js = list(range(fg * ACCG, (fg + 1) * ACCG))
            wts = []
            for ki in range(KT):
                wt = w1p.tile([P, len(js) * P], BF, tag=f"w1_{ki}",
                              name=f"w_w1_{ki}")
                nc.sync.dma_start(
                    out=wt[:], in_=w1d[ki][:, js[0] * P:(js[-1] + 1) * P])
                wts.append(wt)
            yield
            for jj, fj in enumerate(js):
                ps = acc.tile([P, QC], F32, tag="acc", name="acc_ps")
                for ki in range(KT):
                    nc.tensor.matmul(
                        ps[:], lhsT=wts[ki][:, jj * P:(jj + 1) * P],
                        rhs=res2b[ki][:, qsl],
                        start=(ki == 0), stop=(ki == KT - 1))
                    yield
                nc.vector.tensor_scalar(
                    out=hT[fj][:], in0=ps[:], scalar1=b1f[fj][:],
                    scalar2=0.0, op0=ALU.add, op1=ALU.max)
        z3 = [znp.tile([P, QC], F32, tag=f"z_{k}", name="z3")
              for k in range(KT)]
        z3b = [znp.tile([P, QC], BF, tag=f"zb{k}", name="z3b")
               for k in range(KT)]
        w2d = dram("w2")
        for fg in range(KT):
            js = [fg]
            wts = []
            for ki in range(FT):
                wt = w2p.tile([P, P], BF, tag=f"w2_{ki}",
                              name=f"w_w2_{ki}")
                nc.sync.dma_start(
                    out=wt[:], in_=w2d[ki][:, fg * P:(fg + 1) * P])
                wts.append(wt)
            yield
            for jj, fj in enumerate(js):
                ps = acc.tile([P, QC], F32, tag="acc", name="acc_ps")
                for ki in range(FT):
                    nc.tensor.matmul(
                        ps[:], lhsT=wts[ki][:, jj * P:(jj + 1) * P],
                        rhs=hT[ki][:],
                        start=(ki == 0), stop=(ki == FT - 1))
                    yield
                nc.vector.scalar_tensor_tensor(
                    out=z3[fj][:], in0=ps[:], scalar=b2f[fj][:],
                    in1=res2b[fj][:, qsl], op0=ALU.add, op1=ALU.add)
                nc.vector.tensor_copy(out=z3b[fj][:], in_=z3[fj][:])
        outd = dram("outT")
        layer_norm_chunk(
            [t[:] for t in z3], [t[:] for t in z3b], g3, be3,
            out_sl=lambda ki: z3[ki][:],
            out_dram=lambda ki: outd[ki][:, qsl])
        yield

    for qi in range(NQ):
        qsl = slice(qi * QC, (qi + 1) * QC)
        attn_chunk(KTc, QTc, Vsc, NSK, 1, qi, f2, 4)
        norm_chunk(1, qi)
        f2.drain()
        z2 = [znp.tile([P, QC], F32, tag=f"z_{k}", name="z2")
              for k in range(KT)]
        z2b = [znp.tile([P, QC], BF, tag=f"zb{k}", name="z2b")
               for k in range(KT)]

        def ev_o2(fj, tci, ps, qsl=qsl, z2=z2, z2b=z2b):
            nc.vector.scalar_tensor_tensor(
                out=z2[fj][:], in0=ps[:], scalar=INV2,
                in1=res1b[fj][:, qsl], op0=ALU.mult, op1=ALU.add)
            nc.vector.tensor_copy(out=z2b[fj][:], in_=z2[fj][:])

        for _ in projT_dr("wo2", mgv_all, D, ev_o2, "D", csl=(0, qsl)):
            pass
        if DEBUG["stage"] == "ln2" and qi == 0:
            f2.drain()
            dump("dbg_mgc", mg)
            dump("dbg_z2", [t[:] for t in z2])
            dump("dbg_r1", [t[:] for t in res1b])
            es_F.close()
            ctx.close()
            return
        layer_norm_chunk(
            [t[:] for t in z2], [t[:] for t in z2b], g2, be2,
            out_sl=lambda ki: res2b[ki][:, qsl])
        f2.push(ffn_chunk(qi))
    f2.drain()
    es_F.close()

    ctx.close()


# ----------------------------------------------------------------------------
# host glue
# ----------------------------------------------------------------------------

F8NP = ml_dtypes.float8_e4m3
F8MAX = 240.0


def _to_f8(a):
    return np.ascontiguousarray(
        np.clip(np.asarray(a, np.float32), -F8MAX, F8MAX).astype(F8NP))


def _to_bf(a):
    return np.ascontiguousarray(np.asarray(a).astype(ml_dtypes.bfloat16))


def _to_f32(a):
    return np.ascontiguousarray(np.asarray(a).astype(np.float32))


def _pairs_T(xT, KP):
    # x^T [D, L] -> fp8 [KP, P, 2, L]
    Dd, L = xT.shape
    return _to_f8(xT.reshape(KP, 2, P, L).transpose(0, 2, 1, 3))


def _w_pairs(w, scale=WS):
    # W [Din, F] -> fp8 [KP, P, 2, F] (x scale)
    Din, F = w.shape
    return _to_f8((np.asarray(w, np.float32) * scale)
                  .reshape(Din // 256, 2, P, F).transpose(0, 2, 1, 3))


def _prep_weights(inp, D, H, DFF):
    def pp(b):
        return _to_f32(np.asarray(b).reshape(-1, P, 1))

    hidx = np.arange(H)[:, None] * 3 * HD + np.arange(HD)[None, :]
    perm_q = hidx.ravel()
    perm_k = (hidx + HD).ravel()
    perm_v = (hidx + 2 * HD).ravel()
    qkv_w, qkv_b = np.asarray(inp["qkv_w"]), np.asarray(inp["qkv_b"])
    kv_w, kv_b = np.asarray(inp["kv_w"]), np.asarray(inp["kv_b"])
    h2 = np.arange(H)[:, None] * 2 * HD + np.arange(HD)[None, :]
    perm_kc = h2.ravel()
    perm_vc = (h2 + HD).ravel()

    def tile_bf(w):
        w = np.asarray(w)
        return _to_bf(w.reshape(w.shape[0] // P, P, w.shape[1]))

    return dict(
        wq=_w_pairs(qkv_w[:, perm_q]), wk=_w_pairs(qkv_w[:, perm_k]),
        wv=_w_pairs(qkv_w[:, perm_v]),
        bq=pp(qkv_b[perm_q]), bk=pp(qkv_b[perm_k]),
        bv_row=_to_f32(qkv_b[perm_v].reshape(1, D)),
        wo1=_w_pairs(inp["sa_o_w"]),
        wqc=_w_pairs(inp["q_w"]), bqc=pp(inp["q_b"]),
        wkc=_w_pairs(kv_w[:, perm_kc]), bkc=pp(kv_b[perm_kc]),
        wvc=_w_pairs(kv_w[:, perm_vc]),
        bvc_row=_to_f32(kv_b[perm_vc].reshape(1, D)),
        wo2=_w_pairs(inp["ca_o_w"]), bo2=pp(inp["ca_o_b"]),
        w1=tile_bf(inp["ff_w1"]), b1f=pp(inp["ff_b1"]),
        w2=tile_bf(inp["ff_w2"]), b2f=pp(inp["ff_b2"]),
        g1=pp(inp["g1"]), be1=pp(inp["be1"]),
        g2=pp(inp["g2"]), be2=pp(inp["be2"]),
        g3=pp(inp["g3"]), be3=pp(inp["be3"]),
    )


def make_in_maps(inputs, n_cores=8):
    inp = {k: np.asarray(v) for k, v in inputs.items()}
    B, T, D = inp["x_dec"].shape
    S = inp["x_enc"].shape[1]
    DFF = inp["ff_w1"].shape[1]
    H = D // HD
    KP = D // 256
    halves = n_cores // B
    TL = T // halves
    shared = _prep_weights(inp, D, H, DFF)
    sa_o_b = np.asarray(inp["sa_o_b"])
    in_maps = []
    for c in range(n_cores):
        b, half = c // halves, c % halves
        xd = np.asarray(inp["x_dec"][b])       # [T, D]
        xe = np.asarray(inp["x_enc"][b])       # [S, D]
        own = xd[half * TL:(half + 1) * TL]    # [TL, D]
        m = dict(shared)
        m["xdT"] = _pairs_T(xd.T, KP)
        m["xqT"] = _pairs_T(own.T, KP)
        m["xres"] = _to_f32((own + sa_o_b).T.reshape(D // P, P, TL))
        m["xeT"] = _pairs_T(xe.T, KP)
        in_maps.append(m)
    return in_maps, (B, T, D, TL, S, DFF, H, halves)


def assemble_output(results, meta):
    B, T, D, TL, S, DFF, H, halves = meta
    out = np.empty((B, T, D), np.float32)
    for c, r in enumerate(results):
        b, half = c // halves, c % halves
        yT = np.asarray(r["outT"]).reshape(D, TL)
        out[b, half * TL:(half + 1) * TL] = yT.T
    return out


def kernel(**inputs):
    in_maps, meta = make_in_maps(inputs)
    B, T, D, TL, S, DFF, H, halves = meta
    nc = build_program(D=D, H=H, T=T, TL=TL, S=S, DFF=DFF)
    res = run_bass_kernel_spmd(nc, in_maps, core_ids=list(range(len(in_maps))))
    return assemble_output(res.results, meta)
